# revision 2
# baseline (speedup 1.0000x reference)
"""BinNorm (sum-of-sigmoids row normalization via root-find) for Trainium2.

Math: for each row x of shape [256], find nu s.t. sum(sigmoid(x + nu)) == 64,
then output sigmoid(x + nu).  The reference finds nu by a branch-lattice
bisection whose final bracket width is ~6.8e-5 (it quantizes nu to the bracket
midpoint).  Any nu within that quantization radius of the true root produces
outputs within ~1e-5 absmax of the reference, which is below the fp32
reordering noise floor of the reference itself (~1.7e-5).

Kernel algorithm per row:
  1. mean/var via bn_stats -> quadratic-poly initializer nu0 (max err ~0.03)
  2. Newton step  (sigmoid pass + square pass, both with free row-accumulate)
  3. secant step  (one more sigmoid pass)
  4. output pass  sigmoid(x + nu2)
All sigmoid/square evaluations are single ACT instructions over [128, 256]
tiles using the per-partition bias + accum_out features.

Sharding: pure data parallel over rows, 8 cores x 2048 rows.
"""

import numpy as np

_CORES = 8
_B, _D = 16384, 256
_BC = _B // _CORES          # rows per core
_P = 128                    # partitions
_T = _BC // _P              # 16 row-tiles per core
_G = 4                      # tiles per update group
_NG = _T // _G

# nu0 = C0 + C1*m + C2*v + C3*m^2 + C4*m*v + C5*v^2  (m=row mean, v=row var),
# least-squares fit of the true root over N(0,1) rows.
_C = (-1.097386107696299, -1.0174597913968035, -0.24531199751746788,
      0.010321566224828467, 0.005161273657493432, 0.027572120704527067)

_KF = 64.0                  # target sum
_DF_EPS = 1e-6              # secant denominator clamp

_cache: dict = {}


def _build_nc():
    from contextlib import ExitStack
    import concourse.bacc as bacc
    import concourse.mybir as mybir
    import concourse.tile as tile

    f32 = mybir.dt.float32
    SIG = mybir.ActivationFunctionType.Sigmoid
    SQ = mybir.ActivationFunctionType.Square
    A = mybir.AluOpType

    nc = bacc.Bacc(
        "TRN2",
        target_bir_lowering=False,
        debug=False,
        enable_asserts=False,
        num_devices=_CORES,
    )
    x = nc.dram_tensor("x", [_BC, _D], f32, kind="ExternalInput").ap()
    y = nc.dram_tensor("y", [_BC, _D], f32, kind="ExternalOutput").ap()

    with tile.TileContext(nc) as tc, ExitStack() as ctx:
        xp = ctx.enter_context(tc.tile_pool(name="xp", bufs=1))
        sp = ctx.enter_context(tc.tile_pool(name="sp", bufs=8))
        op = ctx.enter_context(tc.tile_pool(name="op", bufs=6))
        st = ctx.enter_context(tc.tile_pool(name="st", bufs=1))

        # resident x tiles, loaded up-front
        xt = []
        for t in range(_T):
            tl = xp.tile([_P, _D], f32, tag=f"x{t}", name=f"x{t}")
            nc.sync.dma_start(tl[:], x[t * _P:(t + 1) * _P, :])
            xt.append(tl)

        def stile(tag, w=_G):
            return st.tile([_P, w], f32, tag=tag, name=tag)

        for g in range(_NG):
            t0 = g * _G

            # ---- moments ----
            agg = st.tile([_P, 2 * _G], f32, tag=f"agg{g}", name=f"agg{g}")
            aggv = agg[:].rearrange("p (c g) -> p c g", g=_G)  # [P,2,G]
            for j in range(_G):
                bn6 = st.tile([_P, 6], f32, tag=f"bn6_{g}_{j}", name=f"bn6_{g}_{j}")
                nc.vector.bn_stats(bn6[:], xt[t0 + j][:])
                nc.vector.bn_aggr(aggv[:, :, j], bn6[:])
            m1 = aggv[:, 0, :]   # [P,G] mean
            vv = aggv[:, 1, :]   # [P,G] var

            # ---- initializer poly ----
            t1 = stile(f"t1_{g}")
            nc.vector.tensor_scalar(t1[:], m1, _C[3], _C[1], A.mult, A.add)
            t2 = stile(f"t2_{g}")
            nc.vector.scalar_tensor_tensor(t2[:], vv, _C[4], t1[:], A.mult, A.add)
            t3 = stile(f"t3_{g}")
            nc.vector.tensor_mul(t3[:], t2[:], m1)
            t4 = stile(f"t4_{g}")
            nc.vector.tensor_scalar(t4[:], vv, _C[5], _C[2], A.mult, A.add)
            t5 = stile(f"t5_{g}")
            nc.vector.tensor_mul(t5[:], t4[:], vv)
            t6 = stile(f"t6_{g}")
            nc.vector.tensor_add(t6[:], t3[:], t5[:])
            nu0 = stile(f"nu0_{g}")
            nc.vector.tensor_scalar_add(nu0[:], t6[:], _C[0])

            # ---- pass 1: sigmoid(x+nu0), accumulate f0 = sum sigmoid ----
            f0 = stile(f"f0_{g}")
            q0 = stile(f"q0_{g}")
            scrs = []
            for j in range(_G):
                scr = sp.tile([_P, _D], f32, tag="scr", name=f"scr_{g}_{j}")
                nc.scalar.activation(scr[:], xt[t0 + j][:], SIG,
                                     bias=nu0[:, j:j + 1],
                                     accum_out=f0[:, j:j + 1])
                scrs.append(scr)
            # ---- pass 2: square(sigmoid), accumulate q0 = sum sigmoid^2 ----
            for j in range(_G):
                scr2 = sp.tile([_P, _D], f32, tag="scr2", name=f"scr2_{g}_{j}")
                nc.scalar.activation(scr2[:], scrs[j][:], SQ,
                                     accum_out=q0[:, j:j + 1])

            # ---- Newton:  nu1 = nu0 - (f0-K)/(f0-q0) ----
            fp = stile(f"fp_{g}")
            nc.vector.tensor_sub(fp[:], f0[:], q0[:])
            rp = stile(f"rp_{g}")
            nc.vector.reciprocal(rp[:], fp[:])
            stp = stile(f"stp_{g}")
            nc.vector.scalar_tensor_tensor(stp[:], f0[:], -_KF, rp[:], A.add, A.mult)
            nu1 = stile(f"nu1_{g}")
            nc.vector.tensor_sub(nu1[:], nu0[:], stp[:])

            # ---- pass 3: sigmoid(x+nu1), accumulate f1 ----
            f1 = stile(f"f1_{g}")
            for j in range(_G):
                scr3 = sp.tile([_P, _D], f32, tag="scr3", name=f"scr3_{g}_{j}")
                nc.scalar.activation(scr3[:], xt[t0 + j][:], SIG,
                                     bias=nu1[:, j:j + 1],
                                     accum_out=f1[:, j:j + 1])

            # ---- secant: nu2 = nu1 - (f1-K)*(nu1-nu0)/clamp(f1-f0) ----
            df = stile(f"df_{g}")
            nc.vector.tensor_sub(df[:], f1[:], f0[:])
            sgn = stile(f"sgn_{g}")
            nc.vector.tensor_scalar(sgn[:], df[:], 0.0, None, A.is_ge)
            off = stile(f"off_{g}")
            nc.vector.tensor_scalar(off[:], sgn[:], 2.0 * _DF_EPS, -_DF_EPS,
                                    A.mult, A.add)
            dfc = stile(f"dfc_{g}")
            nc.vector.tensor_add(dfc[:], df[:], off[:])
            rd = stile(f"rd_{g}")
            nc.vector.reciprocal(rd[:], dfc[:])
            dn = stile(f"dn_{g}")
            nc.vector.tensor_sub(dn[:], nu1[:], nu0[:])
            s2 = stile(f"s2_{g}")
            nc.vector.scalar_tensor_tensor(s2[:], f1[:], -_KF, dn[:], A.add, A.mult)
            s3 = stile(f"s3_{g}")
            nc.vector.tensor_mul(s3[:], s2[:], rd[:])
            nu2 = stile(f"nu2_{g}")
            nc.vector.tensor_sub(nu2[:], nu1[:], s3[:])

            # ---- pass 4: output sigmoid(x+nu2) ----
            for j in range(_G):
                ot = op.tile([_P, _D], f32, tag="out", name=f"ot_{g}_{j}")
                nc.scalar.activation(ot[:], xt[t0 + j][:], SIG,
                                     bias=nu2[:, j:j + 1])
                t = t0 + j
                nc.sync.dma_start(y[t * _P:(t + 1) * _P, :], ot[:])

    nc.compile()
    return nc


def _get_nc():
    if "nc" not in _cache:
        _cache["nc"] = _build_nc()
    return _cache["nc"]


def kernel(x: np.ndarray) -> np.ndarray:
    from concourse.bass_utils import run_bass_kernel_spmd

    x = np.ascontiguousarray(x, dtype=np.float32)
    assert x.shape == (_B, _D), x.shape

    nc = _get_nc()
    in_maps = [{"x": x[i * _BC:(i + 1) * _BC]} for i in range(_CORES)]
    res = run_bass_kernel_spmd(nc, in_maps, list(range(_CORES)))
    out = np.concatenate([res.results[i]["y"] for i in range(_CORES)], axis=0)
    return out.astype(np.float32)


# revision 4
# speedup vs baseline: 1.0577x; 1.0577x over previous
"""BinNorm (sum-of-sigmoids row normalization via root-find) for Trainium2.

Math: for each row x of shape [256], find nu s.t. sum(sigmoid(x + nu)) == 64,
then output sigmoid(x + nu).  The reference finds nu by a branch-lattice
bisection whose final bracket width is ~6.8e-5 (it quantizes nu to the bracket
midpoint).  Any nu within that quantization radius of the true root produces
outputs within ~1e-5 absmax of the reference, which is below the fp32
reordering noise floor of the reference itself (~1.7e-5).

Kernel algorithm per row:
  1. mean/var via bn_stats -> quadratic-poly initializer nu0 (max err ~0.03)
  2. Newton step  (sigmoid pass + square pass, both with free row-accumulate)
  3. secant step  (one more sigmoid pass)
  4. output pass  sigmoid(x + nu2)
All sigmoid/square evaluations are single ACT instructions over [128, 256]
tiles using the per-partition bias + accum_out features.

Sharding: pure data parallel over rows, 8 cores x 2048 rows.
"""

import numpy as np

_CORES = 8
_B, _D = 16384, 256
_BC = _B // _CORES          # rows per core
_P = 128                    # partitions
_T = _BC // _P              # 16 row-tiles per core
_G = 4                      # tiles per update group
_NG = _T // _G

# nu0 = C0 + C1*m + C2*v + C3*m^2 + C4*m*v + C5*v^2  (m=row mean, v=row var),
# least-squares fit of the true root over N(0,1) rows.
_C = (-1.097386107696299, -1.0174597913968035, -0.24531199751746788,
      0.010321566224828467, 0.005161273657493432, 0.027572120704527067)

_KF = 64.0                  # target sum
_DF_EPS = 1e-6              # secant denominator clamp

_cache: dict = {}


def _build_nc():
    from contextlib import ExitStack
    import concourse.bacc as bacc
    import concourse.mybir as mybir
    import concourse.tile as tile

    f32 = mybir.dt.float32
    SIG = mybir.ActivationFunctionType.Sigmoid
    SQ = mybir.ActivationFunctionType.Square
    A = mybir.AluOpType

    nc = bacc.Bacc(
        "TRN2",
        target_bir_lowering=False,
        debug=False,
        enable_asserts=False,
        num_devices=_CORES,
    )
    x = nc.dram_tensor("x", [_BC, _D], f32, kind="ExternalInput").ap()
    y = nc.dram_tensor("y", [_BC, _D], f32, kind="ExternalOutput").ap()

    with tile.TileContext(nc) as tc, ExitStack() as ctx:
        xp = ctx.enter_context(tc.tile_pool(name="xp", bufs=1))
        sp = ctx.enter_context(tc.tile_pool(name="sp", bufs=8))
        op = ctx.enter_context(tc.tile_pool(name="op", bufs=6))
        st = ctx.enter_context(tc.tile_pool(name="st", bufs=1))

        # resident x tiles, loaded up-front
        xt = []
        for t in range(_T):
            tl = xp.tile([_P, _D], f32, tag=f"x{t}", name=f"x{t}")
            nc.sync.dma_start(tl[:], x[t * _P:(t + 1) * _P, :])
            xt.append(tl)

        def stile(tag, w=_G):
            return st.tile([_P, w], f32, tag=tag, name=tag)

        for g in range(_NG):
            t0 = g * _G

            # ---- moments ----
            agg = st.tile([_P, 2 * _G], f32, tag=f"agg{g}", name=f"agg{g}")
            aggv = agg[:].rearrange("p (c g) -> p c g", g=_G)  # [P,2,G]
            for j in range(_G):
                bn6 = st.tile([_P, 6], f32, tag=f"bn6_{g}_{j}", name=f"bn6_{g}_{j}")
                nc.vector.bn_stats(bn6[:], xt[t0 + j][:])
                nc.vector.bn_aggr(aggv[:, :, j], bn6[:])
            m1 = aggv[:, 0, :]   # [P,G] mean
            vv = aggv[:, 1, :]   # [P,G] var

            # ---- initializer poly ----
            t1 = stile(f"t1_{g}")
            nc.vector.tensor_scalar(t1[:], m1, _C[3], _C[1], A.mult, A.add)
            t2 = stile(f"t2_{g}")
            nc.vector.scalar_tensor_tensor(t2[:], vv, _C[4], t1[:], A.mult, A.add)
            t3 = stile(f"t3_{g}")
            nc.vector.tensor_mul(t3[:], t2[:], m1)
            t4 = stile(f"t4_{g}")
            nc.vector.tensor_scalar(t4[:], vv, _C[5], _C[2], A.mult, A.add)
            t5 = stile(f"t5_{g}")
            nc.vector.tensor_mul(t5[:], t4[:], vv)
            t6 = stile(f"t6_{g}")
            nc.vector.tensor_add(t6[:], t3[:], t5[:])
            nu0 = stile(f"nu0_{g}")
            nc.vector.tensor_scalar_add(nu0[:], t6[:], _C[0])

            # Newton iteration nu <- nu - (f-K)/(f-q), where one ACT pass
            # yields sigma elementwise + f = sum sigma (accum), and DVE
            # computes q = sum sigma^2 with a fused multiply-reduce.
            def newton(nuv, it):
                fv = stile(f"f{it}_{g}")
                qv = stile(f"q{it}_{g}")
                for j in range(_G):
                    scr = sp.tile([_P, _D], f32, tag="scr",
                                  name=f"scr_{g}_{it}_{j}")
                    nc.scalar.activation(scr[:], xt[t0 + j][:], SIG,
                                         bias=nuv[:, j:j + 1],
                                         accum_out=fv[:, j:j + 1])
                    sq = sp.tile([_P, _D], f32, tag="sq",
                                 name=f"sq_{g}_{it}_{j}")
                    nc.vector.tensor_mul(sq[:], scr[:], scr[:])
                    nc.vector.tensor_reduce(qv[:, j:j + 1], sq[:],
                                            mybir.AxisListType.X, A.add)
                fp = stile(f"fp{it}_{g}")
                nc.vector.tensor_sub(fp[:], fv[:], qv[:])
                rp = stile(f"rp{it}_{g}")
                nc.vector.reciprocal(rp[:], fp[:])
                stp = stile(f"stp{it}_{g}")
                nc.vector.scalar_tensor_tensor(stp[:], fv[:], -_KF, rp[:],
                                               A.add, A.mult)
                nun = stile(f"nu{it + 1}_{g}")
                nc.vector.tensor_sub(nun[:], nuv[:], stp[:])
                return nun

            nu1 = newton(nu0, 0)
            nu2 = newton(nu1, 1)

            # ---- pass 4: output sigmoid(x+nu2) ----
            for j in range(_G):
                ot = op.tile([_P, _D], f32, tag="out", name=f"ot_{g}_{j}")
                nc.scalar.activation(ot[:], xt[t0 + j][:], SIG,
                                     bias=nu2[:, j:j + 1])
                t = t0 + j
                nc.sync.dma_start(y[t * _P:(t + 1) * _P, :], ot[:])

    nc.compile()
    return nc


def _get_nc():
    if "nc" not in _cache:
        _cache["nc"] = _build_nc()
    return _cache["nc"]


def kernel(x: np.ndarray) -> np.ndarray:
    from concourse.bass_utils import run_bass_kernel_spmd

    x = np.ascontiguousarray(x, dtype=np.float32)
    assert x.shape == (_B, _D), x.shape

    nc = _get_nc()
    in_maps = [{"x": x[i * _BC:(i + 1) * _BC]} for i in range(_CORES)]
    res = run_bass_kernel_spmd(nc, in_maps, list(range(_CORES)))
    out = np.concatenate([res.results[i]["y"] for i in range(_CORES)], axis=0)
    return out.astype(np.float32)


# revision 5
# speedup vs baseline: 1.2110x; 1.1450x over previous
"""BinNorm (sum-of-sigmoids row normalization via root-find) for Trainium2.

Math: for each row x of shape [256], find nu s.t. sum(sigmoid(x + nu)) == 64,
then output sigmoid(x + nu).  The reference finds nu by a branch-lattice
bisection whose final bracket width is ~6.8e-5 (it quantizes nu to the bracket
midpoint).  Any nu within that quantization radius of the true root produces
outputs within ~1e-5 absmax of the reference, which is below the fp32
reordering noise floor of the reference itself (~1.7e-5).

Kernel algorithm per row:
  1. mean/var via bn_stats -> quadratic-poly initializer nu0 (max err ~0.03)
  2. Newton step  (sigmoid pass + square pass, both with free row-accumulate)
  3. secant step  (one more sigmoid pass)
  4. output pass  sigmoid(x + nu2)
All sigmoid/square evaluations are single ACT instructions over [128, 256]
tiles using the per-partition bias + accum_out features.

Sharding: pure data parallel over rows, 8 cores x 2048 rows.
"""

import numpy as np

_CORES = 8
_B, _D = 16384, 256
_BC = _B // _CORES          # rows per core
_P = 128                    # partitions
_T = _BC // _P              # 16 row-tiles per core
_G = 4                      # tiles per update group
_NG = _T // _G

# nu0 = C0 + C1*m + C2*v + C3*m^2 + C4*m*v + C5*v^2  (m=row mean, v=row var),
# least-squares fit of the true root over N(0,1) rows.
_C = (-1.097386107696299, -1.0174597913968035, -0.24531199751746788,
      0.010321566224828467, 0.005161273657493432, 0.027572120704527067)

_KF = 64.0                  # target sum
_DF_EPS = 1e-6              # secant denominator clamp

_cache: dict = {}


def _build_nc():
    from contextlib import ExitStack
    import concourse.bacc as bacc
    import concourse.mybir as mybir
    import concourse.tile as tile

    f32 = mybir.dt.float32
    SIG = mybir.ActivationFunctionType.Sigmoid
    SQ = mybir.ActivationFunctionType.Square
    A = mybir.AluOpType

    nc = bacc.Bacc(
        "TRN2",
        target_bir_lowering=False,
        debug=False,
        enable_asserts=False,
        num_devices=_CORES,
    )
    x = nc.dram_tensor("x", [_BC, _D], f32, kind="ExternalInput").ap()
    y = nc.dram_tensor("y", [_BC, _D], f32, kind="ExternalOutput").ap()

    with tile.TileContext(nc) as tc, ExitStack() as ctx:
        xp = ctx.enter_context(tc.tile_pool(name="xp", bufs=1))
        sp = ctx.enter_context(tc.tile_pool(name="sp", bufs=8))
        op = ctx.enter_context(tc.tile_pool(name="op", bufs=6))
        st = ctx.enter_context(tc.tile_pool(name="st", bufs=1))

        # resident x tiles, loaded up-front
        xt = []
        for t in range(_T):
            tl = xp.tile([_P, _D], f32, tag=f"x{t}", name=f"x{t}")
            nc.sync.dma_start(tl[:], x[t * _P:(t + 1) * _P, :])
            xt.append(tl)

        def stile(tag, w=_G):
            return st.tile([_P, w], f32, tag=tag, name=tag)

        for g in range(_NG):
            t0 = g * _G

            # ---- moments ----
            agg = st.tile([_P, 2 * _G], f32, tag=f"agg{g}", name=f"agg{g}")
            aggv = agg[:].rearrange("p (c g) -> p c g", g=_G)  # [P,2,G]
            for j in range(_G):
                bn6 = st.tile([_P, 6], f32, tag=f"bn6_{g}_{j}", name=f"bn6_{g}_{j}")
                nc.vector.bn_stats(bn6[:], xt[t0 + j][:])
                nc.vector.bn_aggr(aggv[:, :, j], bn6[:])
            m1 = aggv[:, 0, :]   # [P,G] mean
            vv = aggv[:, 1, :]   # [P,G] var

            # ---- initializer poly ----
            t1 = stile(f"t1_{g}")
            nc.vector.tensor_scalar(t1[:], m1, _C[3], _C[1], A.mult, A.add)
            t2 = stile(f"t2_{g}")
            nc.vector.scalar_tensor_tensor(t2[:], vv, _C[4], t1[:], A.mult, A.add)
            t3 = stile(f"t3_{g}")
            nc.vector.tensor_mul(t3[:], t2[:], m1)
            t4 = stile(f"t4_{g}")
            nc.vector.tensor_scalar(t4[:], vv, _C[5], _C[2], A.mult, A.add)
            t5 = stile(f"t5_{g}")
            nc.vector.tensor_mul(t5[:], t4[:], vv)
            t6 = stile(f"t6_{g}")
            nc.vector.tensor_add(t6[:], t3[:], t5[:])
            nu0 = stile(f"nu0_{g}")
            nc.vector.tensor_scalar_add(nu0[:], t6[:], _C[0])

            # Newton step: one ACT pass yields sigma elementwise + f = sum
            # sigma (accum); DVE computes q = sum sigma^2; slope f' = f - q.
            # The second step is a chord step reusing the first step's 1/f'.
            f0 = stile(f"f0_{g}")
            q0 = stile(f"q0_{g}")
            for j in range(_G):
                scr = sp.tile([_P, _D], f32, tag="scr", name=f"scr_{g}_{j}")
                nc.scalar.activation(scr[:], xt[t0 + j][:], SIG,
                                     bias=nu0[:, j:j + 1],
                                     accum_out=f0[:, j:j + 1])
                sq = sp.tile([_P, _D], f32, tag="sq", name=f"sq_{g}_{j}")
                nc.vector.tensor_mul(sq[:], scr[:], scr[:])
                nc.vector.tensor_reduce(q0[:, j:j + 1], sq[:],
                                        mybir.AxisListType.X, A.add)
            fp = stile(f"fp_{g}")
            nc.vector.tensor_sub(fp[:], f0[:], q0[:])
            rp = stile(f"rp_{g}")
            nc.vector.reciprocal(rp[:], fp[:])
            stp = stile(f"stp_{g}")
            nc.vector.scalar_tensor_tensor(stp[:], f0[:], -_KF, rp[:],
                                           A.add, A.mult)
            nu1 = stile(f"nu1_{g}")
            nc.vector.tensor_sub(nu1[:], nu0[:], stp[:])

            f1 = stile(f"f1_{g}")
            for j in range(_G):
                scr3 = sp.tile([_P, _D], f32, tag="scr3", name=f"scr3_{g}_{j}")
                nc.scalar.activation(scr3[:], xt[t0 + j][:], SIG,
                                     bias=nu1[:, j:j + 1],
                                     accum_out=f1[:, j:j + 1])
            stp1 = stile(f"stp1_{g}")
            nc.vector.scalar_tensor_tensor(stp1[:], f1[:], -_KF, rp[:],
                                           A.add, A.mult)
            nu2 = stile(f"nu2_{g}")
            nc.vector.tensor_sub(nu2[:], nu1[:], stp1[:])

            # ---- pass 4: output sigmoid(x+nu2) ----
            for j in range(_G):
                ot = op.tile([_P, _D], f32, tag="out", name=f"ot_{g}_{j}")
                nc.scalar.activation(ot[:], xt[t0 + j][:], SIG,
                                     bias=nu2[:, j:j + 1])
                t = t0 + j
                nc.sync.dma_start(y[t * _P:(t + 1) * _P, :], ot[:])

    nc.compile()
    return nc


def _get_nc():
    if "nc" not in _cache:
        _cache["nc"] = _build_nc()
    return _cache["nc"]


def kernel(x: np.ndarray) -> np.ndarray:
    from concourse.bass_utils import run_bass_kernel_spmd

    x = np.ascontiguousarray(x, dtype=np.float32)
    assert x.shape == (_B, _D), x.shape

    nc = _get_nc()
    in_maps = [{"x": x[i * _BC:(i + 1) * _BC]} for i in range(_CORES)]
    res = run_bass_kernel_spmd(nc, in_maps, list(range(_CORES)))
    out = np.concatenate([res.results[i]["y"] for i in range(_CORES)], axis=0)
    return out.astype(np.float32)


# revision 6
# speedup vs baseline: 1.3754x; 1.1357x over previous
"""BinNorm (sum-of-sigmoids row normalization via root-find) for Trainium2.

Math: for each row x of shape [256], find nu s.t. sum(sigmoid(x + nu)) == 64,
then output sigmoid(x + nu).  The reference finds nu by a branch-lattice
bisection whose final bracket width is ~6.8e-5 (it quantizes nu to the bracket
midpoint).  Any nu within that quantization radius of the true root produces
outputs within ~1e-5 absmax of the reference, below the fp32 reordering noise
floor of the reference itself (~1.7e-5).

Kernel algorithm per row:
  1. mean/var via bn_stats -> quadratic-poly initializer nu0 (max err ~0.03)
  2. Newton step   (sigmoid ACT pass with row-accumulate f; DVE sum sigma^2)
  3. chord step    (one more sigmoid pass, reuse the Newton reciprocal slope)
  4. output pass   sigmoid(x + nu2)
All sigmoid evaluations are single ACT instructions over [128, 256] tiles
using the per-partition bias + accum_out features.

Sharding: pure data parallel over rows, 8 cores x 2048 rows.
"""

import os as _os
import numpy as np

_CORES = 8
_B, _D = 16384, 256
_BC = _B // _CORES          # rows per core
_P = 128                    # partitions
_T = _BC // _P              # 16 row-tiles per core
_G = int(_os.environ.get("BK_G", "2"))       # tiles per update group
_NG = _T // _G
_SCR_BUFS = int(_os.environ.get("BK_SCR_BUFS", "8"))
_STT = _os.environ.get("BK_STT", "0") == "1"  # fused sigma^2 via STT+accum
# input/output DMA block sizes (in 128-row tiles); loads front-loaded small,
# stores tail-loaded small
_IN_BLOCKS = (1, 1, 2, 4, 4, 4)
_OUT_BLOCKS = (4, 4, 4, 2, 1, 1)

# nu0 = C0 + C1*m + C2*v + C3*m^2 + C4*m*v + C5*v^2  (m=row mean, v=row var),
# least-squares fit of the true root over N(0,1) rows.
_C = (-1.097386107696299, -1.0174597913968035, -0.24531199751746788,
      0.010321566224828467, 0.005161273657493432, 0.027572120704527067)

_KF = 64.0                  # target sum

_cache: dict = {}


def _build_nc():
    from contextlib import ExitStack
    import concourse.bacc as bacc
    import concourse.mybir as mybir
    import concourse.tile as tile

    f32 = mybir.dt.float32
    SIG = mybir.ActivationFunctionType.Sigmoid
    A = mybir.AluOpType
    X = mybir.AxisListType.X

    assert sum(_IN_BLOCKS) == _T and sum(_OUT_BLOCKS) == _T

    nc = bacc.Bacc(
        "TRN2",
        target_bir_lowering=False,
        debug=False,
        enable_asserts=False,
        num_devices=_CORES,
    )
    x = nc.dram_tensor("x", [_BC, _D], f32, kind="ExternalInput").ap()
    y = nc.dram_tensor("y", [_BC, _D], f32, kind="ExternalOutput").ap()

    with tile.TileContext(nc) as tc, ExitStack() as ctx:
        xp = ctx.enter_context(tc.tile_pool(name="xp", bufs=1))
        sp = ctx.enter_context(tc.tile_pool(name="sp", bufs=_SCR_BUFS))
        op = ctx.enter_context(tc.tile_pool(name="op", bufs=1))
        st = ctx.enter_context(tc.tile_pool(name="st", bufs=1))

        # warmup: trigger the sigmoid table load before any data arrives
        wz = st.tile([_P, 1], f32, tag="wz", name="wz")
        nc.vector.memset(wz[:], 0.0)
        wo = st.tile([_P, 1], f32, tag="wo", name="wo")
        nc.scalar.activation(wo[:], wz[:], SIG, bias=wz[:])

        # blocked loads: xt[t] are column views into the block tiles
        xt = [None] * _T
        t = 0
        for b, w in enumerate(_IN_BLOCKS):
            blk = xp.tile([_P, w * _D], f32, tag=f"xb{b}", name=f"xb{b}")
            src = x[t * _P:(t + w) * _P, :].rearrange("(t p) d -> p t d", p=_P)
            nc.sync.dma_start(blk[:].rearrange("p (t d) -> p t d", d=_D), src)
            for j in range(w):
                xt[t + j] = blk[:, (j * _D):(j + 1) * _D]
            t += w

        # blocked stores: ot[t] are column views into the out block tiles
        ot = [None] * _T
        oblk = []
        t = 0
        for b, w in enumerate(_OUT_BLOCKS):
            blk = op.tile([_P, w * _D], f32, tag=f"ob{b}", name=f"ob{b}")
            oblk.append((blk, t, w))
            for j in range(w):
                ot[t + j] = blk[:, (j * _D):(j + 1) * _D]
            t += w

        def stile(tag, w=_G):
            return st.tile([_P, w], f32, tag=tag, name=tag)

        for g in range(_NG):
            t0 = g * _G

            # ---- moments ----
            agg = st.tile([_P, 2 * _G], f32, tag=f"agg{g}", name=f"agg{g}")
            aggv = agg[:].rearrange("p (c g) -> p c g", g=_G)  # [P,2,G]
            for j in range(_G):
                bn6 = st.tile([_P, 6], f32, tag=f"bn6_{g}_{j}",
                              name=f"bn6_{g}_{j}")
                nc.vector.bn_stats(bn6[:], xt[t0 + j])
                nc.vector.bn_aggr(aggv[:, :, j], bn6[:])
            m1 = aggv[:, 0, :]   # [P,G] mean
            vv = aggv[:, 1, :]   # [P,G] var

            # ---- initializer poly ----
            t1 = stile(f"t1_{g}")
            nc.vector.tensor_scalar(t1[:], m1, _C[3], _C[1], A.mult, A.add)
            t2 = stile(f"t2_{g}")
            nc.vector.scalar_tensor_tensor(t2[:], vv, _C[4], t1[:], A.mult, A.add)
            t3 = stile(f"t3_{g}")
            nc.vector.tensor_mul(t3[:], t2[:], m1)
            t4 = stile(f"t4_{g}")
            nc.vector.tensor_scalar(t4[:], vv, _C[5], _C[2], A.mult, A.add)
            t5 = stile(f"t5_{g}")
            nc.vector.tensor_mul(t5[:], t4[:], vv)
            t6 = stile(f"t6_{g}")
            nc.vector.tensor_add(t6[:], t3[:], t5[:])
            nu0 = stile(f"nu0_{g}")
            nc.vector.tensor_scalar_add(nu0[:], t6[:], _C[0])

            # ---- Newton step: nu1 = nu0 - (f0-K)/(f0-q0) ----
            f0 = stile(f"f0_{g}")
            q0 = stile(f"q0_{g}")
            for j in range(_G):
                scr = sp.tile([_P, _D], f32, tag="scr", name=f"scr_{g}_{j}")
                nc.scalar.activation(scr[:], xt[t0 + j], SIG,
                                     bias=nu0[:, j:j + 1],
                                     accum_out=f0[:, j:j + 1])
                if _STT:
                    sq = sp.tile([_P, _D], f32, tag="sq", name=f"sq_{g}_{j}")
                    nc.vector.scalar_tensor_tensor(
                        sq[:], scr[:], 0.0, scr[:], A.add, A.mult,
                        accum_out=q0[:, j:j + 1])
                else:
                    sq = sp.tile([_P, _D], f32, tag="sq", name=f"sq_{g}_{j}")
                    nc.vector.tensor_mul(sq[:], scr[:], scr[:])
                    nc.vector.tensor_reduce(q0[:, j:j + 1], sq[:], X, A.add)
            fp = stile(f"fp_{g}")
            nc.vector.tensor_sub(fp[:], f0[:], q0[:])
            rp = stile(f"rp_{g}")
            nc.vector.reciprocal(rp[:], fp[:])
            stp = stile(f"stp_{g}")
            nc.vector.scalar_tensor_tensor(stp[:], f0[:], -_KF, rp[:],
                                           A.add, A.mult)
            nu1 = stile(f"nu1_{g}")
            nc.vector.tensor_sub(nu1[:], nu0[:], stp[:])

            # ---- chord step: nu2 = nu1 - (f1-K)*rp ----
            f1 = stile(f"f1_{g}")
            for j in range(_G):
                scr3 = sp.tile([_P, _D], f32, tag="scr3", name=f"scr3_{g}_{j}")
                nc.scalar.activation(scr3[:], xt[t0 + j], SIG,
                                     bias=nu1[:, j:j + 1],
                                     accum_out=f1[:, j:j + 1])
            stp1 = stile(f"stp1_{g}")
            nc.vector.scalar_tensor_tensor(stp1[:], f1[:], -_KF, rp[:],
                                           A.add, A.mult)
            nu2 = stile(f"nu2_{g}")
            nc.vector.tensor_sub(nu2[:], nu1[:], stp1[:])

            # ---- output pass ----
            for j in range(_G):
                nc.scalar.activation(ot[t0 + j], xt[t0 + j], SIG,
                                     bias=nu2[:, j:j + 1])

        for blk, t0, w in oblk:
            dst = y[t0 * _P:(t0 + w) * _P, :].rearrange("(t p) d -> p t d", p=_P)
            nc.sync.dma_start(dst, blk[:].rearrange("p (t d) -> p t d", d=_D))

    nc.compile()
    return nc


def _get_nc():
    if "nc" not in _cache:
        _cache["nc"] = _build_nc()
    return _cache["nc"]


def kernel(x: np.ndarray) -> np.ndarray:
    from concourse.bass_utils import run_bass_kernel_spmd

    x = np.ascontiguousarray(x, dtype=np.float32)
    assert x.shape == (_B, _D), x.shape

    nc = _get_nc()
    in_maps = [{"x": x[i * _BC:(i + 1) * _BC]} for i in range(_CORES)]
    res = run_bass_kernel_spmd(nc, in_maps, list(range(_CORES)))
    out = np.concatenate([res.results[i]["y"] for i in range(_CORES)], axis=0)
    return out.astype(np.float32)


# revision 12
# speedup vs baseline: 1.4606x; 1.0620x over previous
"""BinNorm (sum-of-sigmoids row normalization via root-find) for Trainium2.

Math: for each row x of shape [256], find nu s.t. sum(sigmoid(x + nu)) == 64,
then output sigmoid(x + nu).  The reference finds nu by a branch-lattice
bisection whose final bracket width is ~6.8e-5 (it quantizes nu to the bracket
midpoint).  Any nu within that quantization radius of the true root produces
outputs within ~1e-5 absmax of the reference, below the fp32 reordering noise
floor of the reference itself (~1.7e-5).

Kernel algorithm per row:
  1. mean/var via bn_stats -> quadratic-poly initializer nu0 (max err ~0.03)
  2. Newton step   (sigmoid ACT pass with row-accumulate f; DVE sum sigma^2)
  3. chord step    (one more sigmoid pass, reuse the Newton reciprocal slope)
  4. output pass   sigmoid(x + nu2), batched per store block: x+nu2 pre-added
     on the idle GPSIMD engine, one wide sigmoid on ACT
Eval sigmoids are single ACT instructions over [128, 256] tiles using the
per-partition bias + accum_out features.

Sharding: pure data parallel over rows, 8 cores x 2048 rows.
"""

import os as _os
import numpy as np

_CORES = 8
_B, _D = 16384, 256
_BC = _B // _CORES          # rows per core
_P = 128                    # partitions
_T = _BC // _P              # 16 row-tiles per core

# per-group tile counts (first groups small to shorten the startup chain)
_GROUPS = tuple(int(v) for v in _os.environ.get(
    "BK_GROUPS", "1,1,1,1,2,2,2,2,2,2").split(","))
_SCR_BUFS = int(_os.environ.get("BK_SCR_BUFS", "16"))
# input/output DMA block sizes (in 128-row tiles); loads front-loaded small,
# stores tail-loaded small.  width>=2 out blocks get a batched output pass.
_IN_BLOCKS = tuple(int(v) for v in _os.environ.get(
    "BK_IN_BLOCKS", "1,1,2,4,4,4").split(","))
_OUT_BLOCKS = tuple(int(v) for v in _os.environ.get(
    "BK_OUT_BLOCKS", "4,4,4,2,1,1").split(","))
_PRE_ENG = _os.environ.get("BK_PRE_ENG", "gpsimd")  # engine for x+nu pre-adds

# nu0 = C0 + C1*m + C2*v + C3*m^2 + C4*m*v + C5*v^2  (m=row mean, v=row var),
# least-squares fit of the true root over N(0,1) rows.
_C = (-1.097386107696299, -1.0174597913968035, -0.24531199751746788,
      0.010321566224828467, 0.005161273657493432, 0.027572120704527067)

_KF = 64.0                  # target sum

_cache: dict = {}


def _build_nc():
    from contextlib import ExitStack
    import concourse.bacc as bacc
    import concourse.mybir as mybir
    import concourse.tile as tile

    f32 = mybir.dt.float32
    SIG = mybir.ActivationFunctionType.Sigmoid
    A = mybir.AluOpType

    assert sum(_IN_BLOCKS) == _T and sum(_OUT_BLOCKS) == _T
    assert sum(_GROUPS) == _T

    nc = bacc.Bacc(
        "TRN2",
        target_bir_lowering=False,
        debug=False,
        enable_asserts=False,
        num_devices=_CORES,
    )
    x = nc.dram_tensor("x", [_BC, _D], f32, kind="ExternalInput").ap()
    y = nc.dram_tensor("y", [_BC, _D], f32, kind="ExternalOutput").ap()

    with tile.TileContext(nc) as tc, ExitStack() as ctx:
        xp = ctx.enter_context(tc.tile_pool(name="xp", bufs=1))
        sp = ctx.enter_context(tc.tile_pool(name="sp", bufs=_SCR_BUFS))
        op = ctx.enter_context(tc.tile_pool(name="op", bufs=1))
        st = ctx.enter_context(tc.tile_pool(name="st", bufs=1))

        pre_eng = nc.gpsimd if _PRE_ENG == "gpsimd" else nc.vector

        # warmup: trigger the sigmoid table load before any data arrives
        wz = st.tile([_P, 1], f32, tag="wz", name="wz")
        nc.vector.memset(wz[:], 0.0)
        wo = st.tile([_P, 1], f32, tag="wo", name="wo")
        nc.scalar.activation(wo[:], wz[:], SIG, bias=wz[:])

        # blocked loads: xt[t] are column views into the block tiles
        xt = [None] * _T
        t = 0
        for b, w in enumerate(_IN_BLOCKS):
            blk = xp.tile([_P, w * _D], f32, tag=f"xb{b}", name=f"xb{b}")
            src = x[t * _P:(t + w) * _P, :].rearrange("(t p) d -> p t d", p=_P)
            nc.sync.dma_start(blk[:].rearrange("p (t d) -> p t d", d=_D), src)
            for j in range(w):
                xt[t + j] = blk[:, (j * _D):(j + 1) * _D]
            t += w

        # out block tiles; a block's output pass is emitted once every tile's
        # nu2 is known (nu2col[t] below)
        oblk = []           # [blk, t0, w]
        t = 0
        for b, w in enumerate(_OUT_BLOCKS):
            blk = op.tile([_P, w * _D], f32, tag=f"ob{b}", name=f"ob{b}")
            oblk.append([blk, t, w])
            t += w

        nu2col = [None] * _T      # per-tile [P,1] view of its group's nu2

        def emit_ready_outputs():
            while oblk and all(nu2col[t] is not None
                               for t in range(oblk[0][1],
                                              oblk[0][1] + oblk[0][2])):
                blk, t0, w = oblk.pop(0)
                if w >= 2:
                    pre = sp.tile([_P, w * _D], f32, tag="pre",
                                  name=f"pre_{t0}")
                    for j in range(w):
                        pre_eng.tensor_scalar_add(
                            pre[:, j * _D:(j + 1) * _D], xt[t0 + j],
                            nu2col[t0 + j])
                    nc.scalar.activation(blk[:], pre[:], SIG)
                else:
                    for j in range(w):
                        nc.scalar.activation(
                            blk[:, j * _D:(j + 1) * _D], xt[t0 + j], SIG,
                            bias=nu2col[t0 + j])
                dst = y[t0 * _P:(t0 + w) * _P, :].rearrange(
                    "(t p) d -> p t d", p=_P)
                nc.sync.dma_start(dst, blk[:].rearrange("p (t d) -> p t d",
                                                        d=_D))

        t0 = 0
        for g, G in enumerate(_GROUPS):
            def stile(tag, w=G):
                return st.tile([_P, w], f32, tag=tag, name=tag)

            # ---- moments ----
            agg = st.tile([_P, 2 * G], f32, tag=f"agg{g}", name=f"agg{g}")
            aggv = agg[:].rearrange("p (c g) -> p c g", g=G)  # [P,2,G]
            for j in range(G):
                bn6 = st.tile([_P, 6], f32, tag=f"bn6_{g}_{j}",
                              name=f"bn6_{g}_{j}")
                nc.vector.bn_stats(bn6[:], xt[t0 + j])
                nc.vector.bn_aggr(aggv[:, :, j], bn6[:])
            m1 = aggv[:, 0, :]   # [P,G] mean
            vv = aggv[:, 1, :]   # [P,G] var

            # ---- initializer poly ----
            t1 = stile(f"t1_{g}")
            nc.vector.tensor_scalar(t1[:], m1, _C[3], _C[1], A.mult, A.add)
            t2 = stile(f"t2_{g}")
            nc.vector.scalar_tensor_tensor(t2[:], vv, _C[4], t1[:], A.mult, A.add)
            t3 = stile(f"t3_{g}")
            nc.vector.tensor_mul(t3[:], t2[:], m1)
            t4 = stile(f"t4_{g}")
            nc.vector.tensor_scalar(t4[:], vv, _C[5], _C[2], A.mult, A.add)
            t5 = stile(f"t5_{g}")
            nc.vector.tensor_mul(t5[:], t4[:], vv)
            t6 = stile(f"t6_{g}")
            nc.vector.tensor_add(t6[:], t3[:], t5[:])
            nu0 = stile(f"nu0_{g}")
            nc.vector.tensor_scalar_add(nu0[:], t6[:], _C[0])

            # ---- Newton step: nu1 = nu0 - (f0-K)/(f0-q0) ----
            f0 = stile(f"f0_{g}")
            q0 = stile(f"q0_{g}")
            for j in range(G):
                scr = sp.tile([_P, _D], f32, tag="scr", name=f"scr_{g}_{j}")
                nc.scalar.activation(scr[:], xt[t0 + j], SIG,
                                     bias=nu0[:, j:j + 1],
                                     accum_out=f0[:, j:j + 1])
                sq = sp.tile([_P, _D], f32, tag="sq", name=f"sq_{g}_{j}")
                nc.vector.scalar_tensor_tensor(
                    sq[:], scr[:], 0.0, scr[:], A.add, A.mult,
                    accum_out=q0[:, j:j + 1])
            fp = stile(f"fp_{g}")
            nc.vector.tensor_sub(fp[:], f0[:], q0[:])
            rp = stile(f"rp_{g}")
            nc.vector.reciprocal(rp[:], fp[:])
            stp = stile(f"stp_{g}")
            nc.vector.scalar_tensor_tensor(stp[:], f0[:], -_KF, rp[:],
                                           A.add, A.mult)
            nu1 = stile(f"nu1_{g}")
            nc.vector.tensor_sub(nu1[:], nu0[:], stp[:])

            # ---- chord step: nu2 = nu1 - (f1-K)*rp ----
            f1 = stile(f"f1_{g}")
            for j in range(G):
                scr3 = sp.tile([_P, _D], f32, tag="scr3", name=f"scr3_{g}_{j}")
                nc.scalar.activation(scr3[:], xt[t0 + j], SIG,
                                     bias=nu1[:, j:j + 1],
                                     accum_out=f1[:, j:j + 1])
            stp1 = stile(f"stp1_{g}")
            nc.vector.scalar_tensor_tensor(stp1[:], f1[:], -_KF, rp[:],
                                           A.add, A.mult)
            nu2 = stile(f"nu2_{g}")
            nc.vector.tensor_sub(nu2[:], nu1[:], stp1[:])

            for j in range(G):
                nu2col[t0 + j] = nu2[:, j:j + 1]
            emit_ready_outputs()
            t0 += G

        assert not oblk

    nc.compile()
    return nc


def _get_nc():
    if "nc" not in _cache:
        _cache["nc"] = _build_nc()
    return _cache["nc"]


def kernel(x: np.ndarray) -> np.ndarray:
    from concourse.bass_utils import run_bass_kernel_spmd

    x = np.ascontiguousarray(x, dtype=np.float32)
    assert x.shape == (_B, _D), x.shape

    nc = _get_nc()
    in_maps = [{"x": x[i * _BC:(i + 1) * _BC]} for i in range(_CORES)]
    res = run_bass_kernel_spmd(nc, in_maps, list(range(_CORES)))
    out = np.concatenate([res.results[i]["y"] for i in range(_CORES)], axis=0)
    return out.astype(np.float32)


# revision 13
# speedup vs baseline: 1.4861x; 1.0174x over previous
"""BinNorm (sum-of-sigmoids row normalization via root-find) for Trainium2.

Math: for each row x of shape [256], find nu s.t. sum(sigmoid(x + nu)) == 64,
then output sigmoid(x + nu).  The reference finds nu by a branch-lattice
bisection whose final bracket width is ~6.8e-5 (it quantizes nu to the bracket
midpoint).  Any nu within that quantization radius of the true root produces
outputs within ~1e-5 absmax of the reference, below the fp32 reordering noise
floor of the reference itself (~1.7e-5).

Kernel algorithm per row:
  1. mean/var via bn_stats -> quadratic-poly initializer nu0 (max err ~0.03)
  2. Newton step   (sigmoid ACT pass with row-accumulate f; DVE sum sigma^2)
  3. chord step    (one more sigmoid pass, reuse the Newton reciprocal slope)
  4. output pass   sigmoid(x + nu2), batched per store block: x+nu2 pre-added
     on the idle GPSIMD engine, one wide sigmoid on ACT
Eval sigmoids are single ACT instructions over [128, 256] tiles using the
per-partition bias + accum_out features.

Sharding: pure data parallel over rows, 8 cores x 2048 rows.
"""

import os as _os
import numpy as np

_CORES = 8
_B, _D = 16384, 256
_BC = _B // _CORES          # rows per core
_P = 128                    # partitions
_T = _BC // _P              # 16 row-tiles per core

# per-group tile counts (first groups small to shorten the startup chain)
_GROUPS = tuple(int(v) for v in _os.environ.get(
    "BK_GROUPS", "1,1,1,1,2,2,2,2,1,1,1,1").split(","))
_SCR_BUFS = int(_os.environ.get("BK_SCR_BUFS", "16"))
# input/output DMA block sizes (in 128-row tiles); loads front-loaded small,
# stores tail-loaded small.  width>=2 out blocks get a batched output pass.
_IN_BLOCKS = tuple(int(v) for v in _os.environ.get(
    "BK_IN_BLOCKS", "1,1,2,4,4,4").split(","))
_OUT_BLOCKS = tuple(int(v) for v in _os.environ.get(
    "BK_OUT_BLOCKS", "4,4,4,2,1,1").split(","))
_PRE_ENG = _os.environ.get("BK_PRE_ENG", "gpsimd")  # engine for x+nu pre-adds

# nu0 = C0 + C1*m + C2*v + C3*m^2 + C4*m*v + C5*v^2  (m=row mean, v=row var),
# least-squares fit of the true root over N(0,1) rows.
_C = (-1.097386107696299, -1.0174597913968035, -0.24531199751746788,
      0.010321566224828467, 0.005161273657493432, 0.027572120704527067)

_KF = 64.0                  # target sum

_cache: dict = {}


def _build_nc():
    from contextlib import ExitStack
    import concourse.bacc as bacc
    import concourse.mybir as mybir
    import concourse.tile as tile

    f32 = mybir.dt.float32
    SIG = mybir.ActivationFunctionType.Sigmoid
    A = mybir.AluOpType

    assert sum(_IN_BLOCKS) == _T and sum(_OUT_BLOCKS) == _T
    assert sum(_GROUPS) == _T

    nc = bacc.Bacc(
        "TRN2",
        target_bir_lowering=False,
        debug=False,
        enable_asserts=False,
        num_devices=_CORES,
    )
    x = nc.dram_tensor("x", [_BC, _D], f32, kind="ExternalInput").ap()
    y = nc.dram_tensor("y", [_BC, _D], f32, kind="ExternalOutput").ap()

    with tile.TileContext(nc) as tc, ExitStack() as ctx:
        xp = ctx.enter_context(tc.tile_pool(name="xp", bufs=1))
        sp = ctx.enter_context(tc.tile_pool(name="sp", bufs=_SCR_BUFS))
        op = ctx.enter_context(tc.tile_pool(name="op", bufs=1))
        st = ctx.enter_context(tc.tile_pool(name="st", bufs=1))

        pre_eng = nc.gpsimd if _PRE_ENG == "gpsimd" else nc.vector

        # warmup: trigger the sigmoid table load before any data arrives
        wz = st.tile([_P, 1], f32, tag="wz", name="wz")
        nc.vector.memset(wz[:], 0.0)
        wo = st.tile([_P, 1], f32, tag="wo", name="wo")
        nc.scalar.activation(wo[:], wz[:], SIG, bias=wz[:])

        # blocked loads: xt[t] are column views into the block tiles
        xt = [None] * _T
        t = 0
        for b, w in enumerate(_IN_BLOCKS):
            blk = xp.tile([_P, w * _D], f32, tag=f"xb{b}", name=f"xb{b}")
            src = x[t * _P:(t + w) * _P, :].rearrange("(t p) d -> p t d", p=_P)
            nc.sync.dma_start(blk[:].rearrange("p (t d) -> p t d", d=_D), src)
            for j in range(w):
                xt[t + j] = blk[:, (j * _D):(j + 1) * _D]
            t += w

        # out block tiles; a block's output pass is emitted once every tile's
        # nu2 is known (nu2col[t] below)
        oblk = []           # [blk, t0, w]
        t = 0
        for b, w in enumerate(_OUT_BLOCKS):
            blk = op.tile([_P, w * _D], f32, tag=f"ob{b}", name=f"ob{b}")
            oblk.append([blk, t, w])
            t += w

        nu2col = [None] * _T      # per-tile [P,1] view of its group's nu2

        def emit_ready_outputs():
            while oblk and all(nu2col[t] is not None
                               for t in range(oblk[0][1],
                                              oblk[0][1] + oblk[0][2])):
                blk, t0, w = oblk.pop(0)
                if w >= 2:
                    pre = sp.tile([_P, w * _D], f32, tag="pre",
                                  name=f"pre_{t0}")
                    for j in range(w):
                        pre_eng.tensor_scalar_add(
                            pre[:, j * _D:(j + 1) * _D], xt[t0 + j],
                            nu2col[t0 + j])
                    nc.scalar.activation(blk[:], pre[:], SIG)
                else:
                    for j in range(w):
                        nc.scalar.activation(
                            blk[:, j * _D:(j + 1) * _D], xt[t0 + j], SIG,
                            bias=nu2col[t0 + j])
                dst = y[t0 * _P:(t0 + w) * _P, :].rearrange(
                    "(t p) d -> p t d", p=_P)
                nc.sync.dma_start(dst, blk[:].rearrange("p (t d) -> p t d",
                                                        d=_D))

        t0 = 0
        for g, G in enumerate(_GROUPS):
            def stile(tag, w=G):
                return st.tile([_P, w], f32, tag=tag, name=tag)

            # ---- moments ----
            agg = st.tile([_P, 2 * G], f32, tag=f"agg{g}", name=f"agg{g}")
            aggv = agg[:].rearrange("p (c g) -> p c g", g=G)  # [P,2,G]
            for j in range(G):
                bn6 = st.tile([_P, 6], f32, tag=f"bn6_{g}_{j}",
                              name=f"bn6_{g}_{j}")
                nc.vector.bn_stats(bn6[:], xt[t0 + j])
                nc.vector.bn_aggr(aggv[:, :, j], bn6[:])
            m1 = aggv[:, 0, :]   # [P,G] mean
            vv = aggv[:, 1, :]   # [P,G] var

            # ---- initializer poly ----
            t1 = stile(f"t1_{g}")
            nc.vector.tensor_scalar(t1[:], m1, _C[3], _C[1], A.mult, A.add)
            t2 = stile(f"t2_{g}")
            nc.vector.scalar_tensor_tensor(t2[:], vv, _C[4], t1[:], A.mult, A.add)
            t3 = stile(f"t3_{g}")
            nc.vector.tensor_mul(t3[:], t2[:], m1)
            t4 = stile(f"t4_{g}")
            nc.vector.tensor_scalar(t4[:], vv, _C[5], _C[2], A.mult, A.add)
            t5 = stile(f"t5_{g}")
            nc.vector.tensor_mul(t5[:], t4[:], vv)
            t6 = stile(f"t6_{g}")
            nc.vector.tensor_add(t6[:], t3[:], t5[:])
            nu0 = stile(f"nu0_{g}")
            nc.vector.tensor_scalar_add(nu0[:], t6[:], _C[0])

            # ---- Newton step: nu1 = nu0 - (f0-K)/(f0-q0) ----
            f0 = stile(f"f0_{g}")
            q0 = stile(f"q0_{g}")
            for j in range(G):
                scr = sp.tile([_P, _D], f32, tag="scr", name=f"scr_{g}_{j}")
                nc.scalar.activation(scr[:], xt[t0 + j], SIG,
                                     bias=nu0[:, j:j + 1],
                                     accum_out=f0[:, j:j + 1])
                sq = sp.tile([_P, _D], f32, tag="sq", name=f"sq_{g}_{j}")
                nc.vector.scalar_tensor_tensor(
                    sq[:], scr[:], 0.0, scr[:], A.add, A.mult,
                    accum_out=q0[:, j:j + 1])
            fp = stile(f"fp_{g}")
            nc.vector.tensor_sub(fp[:], f0[:], q0[:])
            rp = stile(f"rp_{g}")
            nc.vector.reciprocal(rp[:], fp[:])
            stp = stile(f"stp_{g}")
            nc.vector.scalar_tensor_tensor(stp[:], f0[:], -_KF, rp[:],
                                           A.add, A.mult)
            nu1 = stile(f"nu1_{g}")
            nc.vector.tensor_sub(nu1[:], nu0[:], stp[:])

            # ---- chord step: nu2 = nu1 - (f1-K)*rp ----
            f1 = stile(f"f1_{g}")
            for j in range(G):
                scr3 = sp.tile([_P, _D], f32, tag="scr3", name=f"scr3_{g}_{j}")
                nc.scalar.activation(scr3[:], xt[t0 + j], SIG,
                                     bias=nu1[:, j:j + 1],
                                     accum_out=f1[:, j:j + 1])
            stp1 = stile(f"stp1_{g}")
            nc.vector.scalar_tensor_tensor(stp1[:], f1[:], -_KF, rp[:],
                                           A.add, A.mult)
            nu2 = stile(f"nu2_{g}")
            nc.vector.tensor_sub(nu2[:], nu1[:], stp1[:])

            for j in range(G):
                nu2col[t0 + j] = nu2[:, j:j + 1]
            emit_ready_outputs()
            t0 += G

        assert not oblk

    nc.compile()
    return nc


def _get_nc():
    if "nc" not in _cache:
        _cache["nc"] = _build_nc()
    return _cache["nc"]


def kernel(x: np.ndarray) -> np.ndarray:
    from concourse.bass_utils import run_bass_kernel_spmd

    x = np.ascontiguousarray(x, dtype=np.float32)
    assert x.shape == (_B, _D), x.shape

    nc = _get_nc()
    in_maps = [{"x": x[i * _BC:(i + 1) * _BC]} for i in range(_CORES)]
    res = run_bass_kernel_spmd(nc, in_maps, list(range(_CORES)))
    out = np.concatenate([res.results[i]["y"] for i in range(_CORES)], axis=0)
    return out.astype(np.float32)
